# revision 11
# baseline (speedup 1.0000x reference)
"""Dense multi-head attention (S=4096, H=16, D=64) on 8 Trainium2 NeuronCores.

Sharding: heads split across cores (2 heads per core), no cross-core comms.

v4 design. Trace analysis of v2/v3 showed the kernel is bound by the PSUM
write port: every matmul drains its output at 1 fp32 column (<=128
partitions) per cycle @2.4GHz, and per k-tile-pair group the pipeline
drained 4x512 columns (2 QK + 2 PV) = 853ns — exactly the observed period.
QK's drain traffic (all S^2 scores) is irreducible, but PV's is not:

  - PV N-split col-tiling: each k-tile's PV is issued as two concurrent
    M=64 matmuls on different array column-groups — va covers q-cols
    0:256 at output partitions 0-63 (strips 0-1), vb covers q-cols
    256:512 at partitions 64-127 (strips 2-3). Their drains ride
    different column-group buses, so the k-tile drains in 256 cycles
    instead of 512.
  - The softmax denominator no longer rides a ones-column inside PV
    (which forced M=65 and a full-width drain). Instead, every 4 k-tiles
    a batch of 4 col-tiled M=1 ones-matmuls (tile_position (0,32i))
    computes per-k-tile denominator partials concurrently: 512 drain
    cycles per 4 k-tiles. Host sums the 4 partial rows.
  - Port traffic per pair-group: 1024 (QK) + 512 (PV) + 256 (den)
    = 1792 cycles ≈ 747ns vs 853ns before.
  - PSUM: 3x2 banks scores + 1 bank acc + 1 bank den = 8. acc/den are
    single-buffered; the next chunk's first PV lands ~2 periods into the
    chunk, after the epilogue copies have drained them.
  - exp split across ACT (exact, scale folded) and DVE (Schraudolph fp16
    bit-trick) as before; PE pre-warm matmuls + early first-tile DMAs
    hide the HAM cold-clock and DMA-issue ramp.
"""

import os

import numpy as np

import concourse.mybir as mybir
import concourse.tile as tile
from concourse import bacc
from concourse.bass_utils import run_bass_kernel_spmd

S = 4096
H = 16
D = 64
NCORES = 8
HPC = H // NCORES  # heads per core
NKT = S // 128  # 32 k-tiles per head
NPAIR = NKT // 2  # 16 k-tile pairs
NQC = S // 512  # 8 q chunks per head
NCH = 4  # kts/qts load chunks (4 pairs / 1024 q-cols each)
SCALE = 1.0 / np.sqrt(D)

# Schraudolph fp16 exp on DVE: i16 = floor(A*s + B); bitcast to fp16.
DELTA = 0.05
A_CONST = float(1024.0 * np.log2(np.e) * SCALE)
B_CONST = float(15360.0 - 1024.0 * DELTA)

# per-pair exp engine: 'A' = ACT exact exp, 'D' = DVE schraudolph (8/16).
# Last two swapped (D,A) so every chunk's final exp lands on the faster ACT,
# shortening the tail drain of the last chunk.
ENG = ["A", "D", "A", "D", "A", "D", "A", "D", "A", "D", "A", "D", "A", "D", "D", "A"]

F32 = mybir.dt.float32
F16 = mybir.dt.float16
I16 = mybir.dt.int16


def _phase_a(nc, sb, q, k, v, h):
    # ---- Phase A: pure-DMA loads; host ships fp16 in final layouts ----
    # (K^T pair layout, Q^T duplicated, V' with padding columns.)
    qts = [sb.tile([128, 1024], F16, tag=f"qt{b}", name=f"qt{b}") for b in range(NCH)]
    kts = [sb.tile([128, 512], F16, tag=f"kt{b}", name=f"kt{b}") for b in range(NCH)]
    vstage = sb.tile([128, NKT, 128], F16, tag="vstage")

    def load_v_quarter(qt):
        t0, t1 = qt * (NKT // 4), (qt + 1) * (NKT // 4)
        nc.sync.dma_start(
            vstage[:, t0:t1, :],
            v.ap()[h].rearrange("p (n c) -> p n c", c=128)[:, t0:t1],
        )

    # DMA order = need order. The first QK pair of head 0 only needs
    # kts[0][:, 0:128] + qts[0][:, 0:512]; issue those as the two leading
    # tiny DMAs so the PE can start ~2us earlier (DMA descriptor issue on
    # the Sync queue is ~650ns each, serialized).
    if h == 0:
        nc.sync.dma_start(kts[0][:, 0:128], k.ap()[h, :, 0:128])
        nc.sync.dma_start(qts[0][:, 0:512], q.ap()[h, :, 0:512])
        nc.sync.dma_start(kts[0][:, 128:512], k.ap()[h, :, 128:512])
    else:
        nc.sync.dma_start(kts[0][:], k.ap()[h, :, 0:512])
        nc.sync.dma_start(qts[0][:, 0:512], q.ap()[h, :, 0:512])
    load_v_quarter(0)
    nc.sync.dma_start(qts[0][:, 512:1024], q.ap()[h, :, 512:1024])
    load_v_quarter(1)
    nc.sync.dma_start(kts[1][:], k.ap()[h, :, 512:1024])
    nc.sync.dma_start(kts[2][:], k.ap()[h, :, 1024:1536])
    nc.sync.dma_start(kts[3][:], k.ap()[h, :, 1536:2048])
    load_v_quarter(2)
    load_v_quarter(3)
    nc.sync.dma_start(qts[1][:], q.ap()[h, :, 1024:2048])
    nc.sync.dma_start(qts[2][:], q.ap()[h, :, 2048:3072])
    nc.sync.dma_start(qts[3][:], q.ap()[h, :, 3072:4096])
    return qts, kts, vstage


def _phase_b(nc, pools, tiles, ones, zeros, o1, o2):
    sb, epool, spsum, accp, denp = pools
    z128, zmov = zeros

    # ---- Phase B: attention, software-pipelined two pairs deep ----
    qk_n = 256 if os.environ.get("QK_TIMING_MUTANT") else 512

    def qk_pair(h, qc, p):
        qts, kts, _ = tiles[h]
        off = (qc % 2) * 512
        b, j = p // 4, p % 4
        sp = spsum.tile([128, 1024], F32, tag="sp")
        nc.tensor.matmul(
            sp[:, 0:qk_n],
            kts[b][0:64, j * 128 : (j + 1) * 128],
            qts[qc // 2][0:64, off : off + qk_n],
            tile_position=(0, 0),
        )
        nc.tensor.matmul(
            sp[:, 512 : 512 + qk_n],
            kts[b][64:128, j * 128 : (j + 1) * 128],
            qts[qc // 2][64:128, off : off + qk_n],
            tile_position=(64, 0),
        )
        return sp

    def exp_pair(p, sp):
        et = epool.tile([128, 1024], F16, tag="et")
        if ENG[p] == "A":
            nc.scalar.activation(
                et[:], sp[:], mybir.ActivationFunctionType.Exp, scale=SCALE
            )
        else:
            nc.vector.tensor_scalar(
                et[:].bitcast(I16),
                sp[:],
                A_CONST,
                B_CONST,
                op0=mybir.AluOpType.mult,
                op1=mybir.AluOpType.add,
            )
        return et

    def pv_pair(h, p, et, acc):
        # N-split col-tiled PV: per k-tile, two concurrent M=64 matmuls on
        # disjoint column-groups drain on different buses (256 cycles/k-tile).
        # acc quadrants: [0:64, 0:256] = dims x q(0:256),
        #                [64:128, 256:512] = dims x q(256:512).
        # First matmul of the chunk start=True marks the whole bank
        # pending-zero; the partner overwrites its own (still-pending) bytes
        # with start=False. Only the very last matmul carries stop=True.
        vstage = tiles[h][2]
        for s in range(2):
            t = 2 * p + s
            nc.tensor.matmul(
                acc[0:64, 0:256],
                vstage[:, t, 0:64],
                et[:, s * 512 : s * 512 + 256],
                start=False,
                stop=False,
                tile_position=(0, 0),
            )
            nc.tensor.matmul(
                acc[64:128, 256:512],
                vstage[:, t, 0:64],
                et[:, s * 512 + 256 : s * 512 + 512],
                start=False,
                stop=(t == NKT - 1),
                tile_position=(0, 64),
            )

    def den_batch(et_a, et_b, den, first, last):
        # 4 col-tiled M=1 ones-matmuls: denominator partials for 4 k-tiles
        # (2 pairs) drain concurrently on the 4 column-group buses.
        # Row 32*i accumulates k-tiles with t%4 == i; host sums the 4 rows.
        for i in range(4):
            et = et_a if i < 2 else et_b
            s = i % 2
            nc.tensor.matmul(
                den[32 * i : 32 * i + 1, :],
                ones[:, :],
                et[:, s * 512 : (s + 1) * 512],
                start=False,
                stop=(last and i == 3),
                tile_position=(0, 32 * i),
            )

    def epilogue(h, qc, acc, den):
        # Ship the acc bank (2 valid quadrants) + den partial rows; the
        # host assembles num/den. ACT copies keep DVE free.
        fnum = sb.tile([128, 512], F16, tag="fnum")
        fden = sb.tile([128, 512], F16, tag="fden")
        nc.scalar.copy(fnum[:], acc[:])
        nc.scalar.copy(fden[0:97, :], den[0:97, :])
        nc.sync.dma_start(o1.ap()[h, qc], fnum[:])
        nc.sync.dma_start(o2.ap()[h, qc], fden[0:97, :])

    # Flattened pair pipeline across all chunks AND heads: every pair gets
    # 2-3 slots of exp slack, including at chunk and head boundaries.
    NG = HPC * NQC * NPAIR
    accs = {}
    dens = {}

    def hqp(g):
        return g // (NQC * NPAIR), (g // NPAIR) % NQC, g % NPAIR

    def qk_g(g):
        h, qc, p = hqp(g)
        return qk_pair(h, qc, p)

    sps = [qk_g(0), qk_g(1), qk_g(2)]
    ets = [exp_pair(0, sps[0]), exp_pair(1, sps[1])]
    for g in range(NG):
        h, qc, p = hqp(g)
        if g + 2 < NG:
            ets.append(exp_pair((g + 2) % NPAIR, sps[g + 2]))
        if g + 3 < NG:
            sps.append(qk_g(g + 3))
        if p == 0:
            accs[qc] = accp.tile([128, 512], F32, tag="acc", name=f"acc{h}_{qc}")
            dens[qc] = denp.tile([128, 512], F32, tag="den", name=f"den{h}_{qc}")
            # Zero-fill both banks with a zeros-stationary matmul so every
            # real matmul can accumulate with start=False. This is correct
            # regardless of whether the HW's first-matmul clear is bank-wide
            # or per-element (the col-tiled quadrants share one bank).
            nc.tensor.matmul(accs[qc][:, :], z128[:], zmov[:], start=True, stop=False)
            nc.tensor.matmul(dens[qc][:, :], z128[:], zmov[:], start=True, stop=False)
        pv_pair(h, p, ets[g], accs[qc])
        if p % 2 == 1:
            den_batch(
                ets[g - 1], ets[g], dens[qc], first=(p == 1), last=(p == NPAIR - 1)
            )
        if p == NPAIR - 1:
            epilogue(h, qc, accs.pop(qc), dens.pop(qc))


def _build():
    nc = bacc.Bacc(trn_type="TRN2", debug=False, num_devices=NCORES)
    q = nc.dram_tensor("q", [HPC, 128, S], F16, kind="ExternalInput")
    k = nc.dram_tensor("k", [HPC, 128, S // 2], F16, kind="ExternalInput")
    v = nc.dram_tensor("v", [HPC, 128, NKT * 128], F16, kind="ExternalInput")
    o1 = nc.dram_tensor("o1", [HPC, NQC, 128, 512], F16, kind="ExternalOutput")
    o2 = nc.dram_tensor("o2", [HPC, NQC, 97, 512], F16, kind="ExternalOutput")

    with tile.TileContext(nc) as tc:
        with (
            tc.tile_pool(name="const", bufs=1) as cpool,
            tc.tile_pool(name="sb", bufs=2) as sb,
            tc.tile_pool(name="epool", bufs=5) as epool,
            tc.tile_pool(name="spsum", bufs=3, space="PSUM") as spsum,
            tc.tile_pool(name="accp", bufs=1, space="PSUM") as accp,
            tc.tile_pool(name="denp", bufs=1, space="PSUM") as denp,
        ):
            # Dummy exp pulls the ACT table-load DMA ahead of the input DMAs.
            warm = cpool.tile([128, 1], F32, tag="warm")
            nc.gpsimd.memset(warm[:], 0.0)
            nc.scalar.activation(warm[:], warm[:], mybir.ActivationFunctionType.Exp)
            ones = cpool.tile([128, 1], F16, tag="ones")
            nc.gpsimd.memset(ones[:], 1.0)
            # PE pre-warm: ~2us of dummy matmuls during the DMA ramp flips the
            # HAM clock gate to 8/8 before the first real QK. No DMA
            # dependency: operands are memset const tiles; output goes to the
            # first sp-pool buffer (recycled by the real pipeline afterwards).
            wq = cpool.tile([64, 128], F16, tag="wq")
            wx = cpool.tile([64, 512], F16, tag="wx")
            nc.gpsimd.memset(wq[:], 0.0)
            nc.gpsimd.memset(wx[:], 0.0)
            z128 = cpool.tile([128, 128], F16, tag="z128")
            zmov = cpool.tile([128, 512], F16, tag="zmov")
            nc.gpsimd.memset(z128[:], 0.0)
            nc.gpsimd.memset(zmov[:], 0.0)
            warmps = spsum.tile([128, 1024], F32, tag="sp", name="warmps")
            for wi in range(5):
                nc.tensor.matmul(
                    warmps[:, (wi % 2) * 512 : (wi % 2) * 512 + 512],
                    wq[:],
                    wx[:],
                    start=True,
                    stop=True,
                )
            pools = (sb, epool, spsum, accp, denp)
            tiles = [_phase_a(nc, sb, q, k, v, h) for h in range(HPC)]
            _phase_b(nc, pools, tiles, ones, (z128, zmov), o1, o2)

    nc.compile()
    return nc


_NC_CACHE = None


def _prep_inputs(query, key, value, c):
    sl = slice(c * HPC, (c + 1) * HPC)
    f16 = np.float16
    # [S, HPC, D] -> per-head Q^T/K^T [HPC, D, S]
    qh = query[:, sl, :].transpose(1, 2, 0).astype(f16)
    kh = key[:, sl, :].transpose(1, 2, 0).astype(f16)
    # Q^T duplicated on both partition halves: [HPC, 128, S]
    q_dup = np.concatenate([qh, qh], axis=1)
    # K^T pair layout: even k-tiles on rows 0-63, odd on 64-127: [HPC,128,S/2]
    kt = kh.reshape(HPC, D, NKT, 128)
    k_pair = np.concatenate([kt[:, :, 0::2, :], kt[:, :, 1::2, :]], axis=1).reshape(
        HPC, 128, S // 2
    )
    # V' layout [HPC, 128, NKT*128]: vstage[p, t, 0:64] = V[t*128+p, :],
    # cols 64.. = 0 (padding; denominator now comes from ones-matmuls).
    vh = value[:, sl, :].transpose(1, 0, 2).astype(f16)  # [HPC, S, D]
    vp = np.zeros((HPC, NKT, 128, 128), dtype=f16)
    vp[:, :, :, 0:D] = vh.reshape(HPC, NKT, 128, D)
    v_pack = vp.transpose(0, 2, 1, 3).reshape(HPC, 128, NKT * 128)
    return {
        "q": np.ascontiguousarray(q_dup),
        "k": np.ascontiguousarray(k_pair),
        "v": np.ascontiguousarray(v_pack),
    }


def kernel(query, key, value):
    global _NC_CACHE
    if _NC_CACHE is None:
        _NC_CACHE = _build()
    nc = _NC_CACHE

    query = np.asarray(query)
    key = np.asarray(key)
    value = np.asarray(value)
    in_maps = [_prep_inputs(query, key, value, c) for c in range(NCORES)]

    res = run_bass_kernel_spmd(nc, in_maps, core_ids=list(range(NCORES)))
    outs = []
    for c in range(NCORES):
        o1 = res.results[c]["o1"].astype(np.float32)  # [HPC, NQC, 128, 512]
        o2 = res.results[c]["o2"].astype(np.float32)  # [HPC, NQC, 97, 512]
        # num quadrants: [0:64, 0:256] = dims x q(0:256),
        #                [64:128, 256:512] = dims x q(256:512)
        num = np.concatenate(
            [o1[:, :, 0:64, 0:256], o1[:, :, 64:128, 256:512]], axis=3
        )  # [HPC, NQC, 64, 512]
        den = o2[:, :, [0, 32, 64, 96], :].sum(axis=2)  # [HPC, NQC, 512]
        outc = num / den[:, :, None, :]  # [HPC, NQC, D, 512]
        outs.append(outc.transpose(1, 3, 0, 2).reshape(S, HPC, D))
    return np.concatenate(outs, axis=1)


# revision 12
# speedup vs baseline: 1.0705x; 1.0705x over previous
"""Dense multi-head attention (S=4096, H=16, D=64) on 8 Trainium2 NeuronCores.

Sharding: heads split across cores (2 heads per core), no cross-core comms.

v2 design (vs baseline): attacks the two co-bottlenecks (ACT exp at ~270us
busy, PE at ~219us) simultaneously:

  - QK^T uses PE row-tiling: contraction is d=64, so two k-tiles run
    CONCURRENTLY in the 128x128 array as (64,0)/(0,0) row tiles -> ~2x QK
    throughput on HW. Host pre-arranges K^T with even k-tiles on SBUF
    partitions 0-63 and odd k-tiles on 64-127; Q^T is duplicated on both
    partition halves.
  - exp is split across TWO engines: ACT computes exact exp on 8/16 of the
    k-tile pairs; DVE computes a Schraudolph-style fp16 exp on the other
    8/16 (i16 = floor(s*1024*log2e/8 + (15360 - 1024*delta)), bits
    reinterpreted as fp16 -> piecewise-linear e^x, ~3% weight error on the
    offloaded fraction; end metric ~1e-2 vs the 2e-2 gate).
  - PV: stationary V' [128k x 128] fp16 with a ones column at col 64
    (softmax denominator accumulates in output row 64), moving E [128,512]
    fp16, accumulated over all 32 k-tiles into one PSUM bank.
  - Epilogue: no PE transpose and no in-kernel divide. The [65,512]
    numerator^T+denominator block is copied PSUM->SBUF (fp16) on ACT and
    DMA'd out in [D+1, S] layout; the HOST divides and transposes back.
  - The pair pipeline is flattened across q-chunks (2-3 pair-slots of exp
    slack everywhere). The host ships fp16 inputs pre-packed in their final
    SBUF layouts, so Phase A is pure DMA (no on-chip casts or memsets) at
    half the fp32 byte count.
"""

import os

import numpy as np

import concourse.mybir as mybir
import concourse.tile as tile
from concourse import bacc
from concourse.bass_utils import run_bass_kernel_spmd

S = 4096
H = 16
D = 64
NCORES = 8
HPC = H // NCORES  # heads per core
NKT = S // 128  # 32 k-tiles per head
NPAIR = NKT // 2  # 16 k-tile pairs (even/odd row-tiled together)
NQC = S // 512  # 8 q chunks per head
NCH = 4  # kts/qts load chunks (4 pairs / 1024 q-cols each)
SCALE = 1.0 / np.sqrt(D)

# Schraudolph fp16 exp on DVE: i16 = floor(A*s + B); bitcast to fp16.
DELTA = 0.05
A_CONST = float(1024.0 * np.log2(np.e) * SCALE)
B_CONST = float(15360.0 - 1024.0 * DELTA)

# per-pair exp engine: 'A' = ACT exact exp, 'D' = DVE schraudolph (8/16).
# Last two swapped (D,A) so every chunk's final exp lands on the faster ACT,
# shortening the tail drain of the last chunk.
ENG = ["A", "D", "A", "D", "A", "D", "A", "D", "A", "D", "A", "D", "A", "D", "D", "A"]

F32 = mybir.dt.float32
F16 = mybir.dt.float16
I16 = mybir.dt.int16


def _phase_a(nc, sb, q, k, v, h):
    # ---- Phase A: pure-DMA loads; host ships fp16 in final layouts ----
    # (K^T pair layout, Q^T duplicated, V' padded with ones column.)
    qts = [sb.tile([128, 1024], F16, tag=f"qt{b}", name=f"qt{b}") for b in range(NCH)]
    kts = [sb.tile([128, 512], F16, tag=f"kt{b}", name=f"kt{b}") for b in range(NCH)]
    vstage = sb.tile([128, NKT, 128], F16, tag="vstage")

    def load_v_quarter(qt):
        t0, t1 = qt * (NKT // 4), (qt + 1) * (NKT // 4)
        nc.sync.dma_start(
            vstage[:, t0:t1, :],
            v.ap()[h].rearrange("p (n c) -> p n c", c=128)[:, t0:t1],
        )

    # DMA order = need order. The first QK pair of head 0 only needs
    # kts[0][:, 0:128] + qts[0][:, 0:512]; issue those as the two leading
    # tiny DMAs so the PE can start ~2us earlier (DMA descriptor issue on
    # the Sync queue is ~650ns each, serialized).
    if h == 0:
        nc.sync.dma_start(kts[0][:, 0:128], k.ap()[h, :, 0:128])
        nc.sync.dma_start(qts[0][:, 0:512], q.ap()[h, :, 0:512])
        nc.sync.dma_start(kts[0][:, 128:512], k.ap()[h, :, 128:512])
    else:
        nc.sync.dma_start(kts[0][:], k.ap()[h, :, 0:512])
        nc.sync.dma_start(qts[0][:, 0:512], q.ap()[h, :, 0:512])
    load_v_quarter(0)
    nc.sync.dma_start(qts[0][:, 512:1024], q.ap()[h, :, 512:1024])
    load_v_quarter(1)
    nc.sync.dma_start(kts[1][:], k.ap()[h, :, 512:1024])
    nc.sync.dma_start(kts[2][:], k.ap()[h, :, 1024:1536])
    nc.sync.dma_start(kts[3][:], k.ap()[h, :, 1536:2048])
    load_v_quarter(2)
    load_v_quarter(3)
    nc.sync.dma_start(qts[1][:], q.ap()[h, :, 1024:2048])
    nc.sync.dma_start(qts[2][:], q.ap()[h, :, 2048:3072])
    nc.sync.dma_start(qts[3][:], q.ap()[h, :, 3072:4096])
    return qts, kts, vstage


def _phase_b(nc, pools, tiles, o):
    sb, epool, spsum, opsum = pools

    # ---- Phase B: attention, software-pipelined two pairs deep ----
    # _QK_HALF: timing-mutant mode for sim analysis only — issue QK at
    # N=256 (half stream cost) to approximate HW row-tile concurrency,
    # which the cost model does not simulate. WRONG RESULTS; timing only.
    qk_n = 256 if os.environ.get("QK_TIMING_MUTANT") else 512

    def qk_pair(h, qc, p):
        qts, kts, _ = tiles[h]
        off = (qc % 2) * 512
        b, j = p // 4, p % 4
        sp = spsum.tile([128, 1024], F32, tag="sp")
        nc.tensor.matmul(
            sp[:, 0:qk_n],
            kts[b][0:64, j * 128 : (j + 1) * 128],
            qts[qc // 2][0:64, off : off + qk_n],
            tile_position=(0, 0),
        )
        nc.tensor.matmul(
            sp[:, 512 : 512 + qk_n],
            kts[b][64:128, j * 128 : (j + 1) * 128],
            qts[qc // 2][64:128, off : off + qk_n],
            tile_position=(64, 0),
        )
        return sp

    def exp_pair(p, sp):
        et = epool.tile([128, 1024], F16, tag="et")
        if ENG[p] == "A":
            nc.scalar.activation(
                et[:], sp[:], mybir.ActivationFunctionType.Exp, scale=SCALE
            )
        else:
            nc.vector.tensor_scalar(
                et[:].bitcast(I16),
                sp[:],
                A_CONST,
                B_CONST,
                op0=mybir.AluOpType.mult,
                op1=mybir.AluOpType.add,
            )
        return et

    def pv_pair(h, p, et, acc):
        vstage = tiles[h][2]
        for side in range(2):
            t = 2 * p + side
            nc.tensor.matmul(
                acc[:],
                vstage[:, t, :],
                et[:, side * 512 : (side + 1) * 512],
                start=(t == 0),
                stop=(t == NKT - 1),
            )

    def epilogue(h, acc, qs):
        # Ship unnormalized numerator rows 0..63 + denominator row 64;
        # the host divides. (ACT copy PSUM->SBUF keeps DVE free, then DMA.)
        fin = sb.tile([D + 1, 512], F16, tag="fin")
        nc.scalar.copy(fin[:], acc[0 : D + 1, :])
        nc.sync.dma_start(o.ap()[h, :, qs : qs + 512], fin[:])

    # Flattened pair pipeline across all chunks AND heads: every pair gets
    # 2-3 slots of exp slack, including at chunk and head boundaries.
    NG = HPC * NQC * NPAIR
    accs = {}

    def hqp(g):
        return g // (NQC * NPAIR), (g // NPAIR) % NQC, g % NPAIR

    def qk_g(g):
        h, qc, p = hqp(g)
        return qk_pair(h, qc, p)

    sps = [qk_g(0), qk_g(1), qk_g(2)]
    ets = [exp_pair(0, sps[0]), exp_pair(1, sps[1])]
    for g in range(NG):
        h, qc, p = hqp(g)
        if g + 2 < NG:
            ets.append(exp_pair((g + 2) % NPAIR, sps[g + 2]))
        if g + 3 < NG:
            sps.append(qk_g(g + 3))
        if p == 0:
            accs[qc] = opsum.tile([128, 512], F32, tag="acc", name=f"acc{h}_{qc}")
        pv_pair(h, p, ets[g], accs[qc])
        if p == NPAIR - 1:
            epilogue(h, accs.pop(qc), qc * 512)


def _build():
    nc = bacc.Bacc(trn_type="TRN2", debug=False, num_devices=NCORES)
    q = nc.dram_tensor("q", [HPC, 128, S], F16, kind="ExternalInput")
    k = nc.dram_tensor("k", [HPC, 128, S // 2], F16, kind="ExternalInput")
    v = nc.dram_tensor("v", [HPC, 128, NKT * 128], F16, kind="ExternalInput")
    o = nc.dram_tensor("o", [HPC, D + 1, S], F16, kind="ExternalOutput")

    with tile.TileContext(nc) as tc:
        with (
            tc.tile_pool(name="const", bufs=1) as cpool,
            tc.tile_pool(name="sb", bufs=2) as sb,
            tc.tile_pool(name="epool", bufs=4) as epool,
            tc.tile_pool(name="spsum", bufs=3, space="PSUM") as spsum,
            tc.tile_pool(name="opsum", bufs=2, space="PSUM") as opsum,
        ):
            # Dummy exp pulls the ACT table-load DMA ahead of the input DMAs.
            warm = cpool.tile([128, 1], F32, tag="warm")
            nc.gpsimd.memset(warm[:], 0.0)
            nc.scalar.activation(warm[:], warm[:], mybir.ActivationFunctionType.Exp)
            # PE pre-warm: ~2us of dummy matmuls during the DMA ramp flips the
            # HAM clock gate to 8/8 before the first real QK, and keeps the
            # array busy so it never re-throttles. No DMA dependency: operands
            # are memset const tiles; output goes to the first sp-pool buffer
            # (recycled by the real pipeline afterwards).
            wq = cpool.tile([64, 128], F16, tag="wq")
            wx = cpool.tile([64, 512], F16, tag="wx")
            nc.gpsimd.memset(wq[:], 0.0)
            nc.gpsimd.memset(wx[:], 0.0)
            warmps = spsum.tile([128, 1024], F32, tag="sp", name="warmps")
            for wi in range(5):
                nc.tensor.matmul(
                    warmps[:, (wi % 2) * 512 : (wi % 2) * 512 + 512],
                    wq[:],
                    wx[:],
                    start=True,
                    stop=True,
                )
            pools = (sb, epool, spsum, opsum)
            tiles = [_phase_a(nc, sb, q, k, v, h) for h in range(HPC)]
            _phase_b(nc, pools, tiles, o)

    nc.compile()
    return nc


_NC_CACHE = None


def _prep_inputs(query, key, value, c):
    sl = slice(c * HPC, (c + 1) * HPC)
    f16 = np.float16
    # [S, HPC, D] -> per-head Q^T/K^T [HPC, D, S]
    qh = query[:, sl, :].transpose(1, 2, 0).astype(f16)
    kh = key[:, sl, :].transpose(1, 2, 0).astype(f16)
    # Q^T duplicated on both partition halves: [HPC, 128, S]
    q_dup = np.concatenate([qh, qh], axis=1)
    # K^T pair layout: even k-tiles on rows 0-63, odd on 64-127: [HPC,128,S/2]
    kt = kh.reshape(HPC, D, NKT, 128)
    k_pair = np.concatenate([kt[:, :, 0::2, :], kt[:, :, 1::2, :]], axis=1).reshape(
        HPC, 128, S // 2
    )
    # V' layout [HPC, 128, NKT*128]: vstage[p, t, 0:64] = V[t*128+p, :],
    # col 64 = 1.0 (denominator ones), cols 65.. = 0 (FWL padding).
    vh = value[:, sl, :].transpose(1, 0, 2).astype(f16)  # [HPC, S, D]
    vp = np.zeros((HPC, NKT, 128, 128), dtype=f16)
    vp[:, :, :, 0:D] = vh.reshape(HPC, NKT, 128, D)
    vp[:, :, :, D] = f16(1.0)
    v_pack = vp.transpose(0, 2, 1, 3).reshape(HPC, 128, NKT * 128)
    return {
        "q": np.ascontiguousarray(q_dup),
        "k": np.ascontiguousarray(k_pair),
        "v": np.ascontiguousarray(v_pack),
    }


def kernel(query, key, value):
    global _NC_CACHE
    if _NC_CACHE is None:
        _NC_CACHE = _build()
    nc = _NC_CACHE

    query = np.asarray(query)
    key = np.asarray(key)
    value = np.asarray(value)
    in_maps = [_prep_inputs(query, key, value, c) for c in range(NCORES)]

    res = run_bass_kernel_spmd(nc, in_maps, core_ids=list(range(NCORES)))
    # o is [HPC, D+1, S] per core: rows 0..63 numerator^T, row 64 denominator.
    outs = []
    for c in range(NCORES):
        oc = res.results[c]["o"].astype(np.float32)  # [HPC, D+1, S] (fp16 wire)
        num = oc[:, 0:D, :]
        den = oc[:, D : D + 1, :]
        outs.append((num / den).transpose(2, 0, 1))  # [S, HPC, D]
    return np.concatenate(outs, axis=1)



# revision 13
# speedup vs baseline: 1.1799x; 1.1022x over previous
"""Dense multi-head attention (S=4096, H=16, D=64) on 8 Trainium2 NeuronCores.

Sharding: heads split across cores (2 heads per core), no cross-core comms.

v5 = v3 + 2-q-chunk weight batching. Trace analysis of v3 showed the steady
state is fill+transition bound: per pair-group the PE spends 216 (QK pair
fill) + 106 (weight-set swap bubble) + 213 (PV1) + 13 + 213 (PV2) + 99
(swap bubble) = 860ns. The ~100ns bubbles appear exactly at the
QK-weights->V-weights and V-weights->QK-weights transitions. Processing TWO
q-chunks per k-tile pair halves the number of weight-set transitions (and
K/V LDWEIGHTS): per super-group the PE runs QK(j0),QK(j1) on one K weight
set, then PV-even(j0),PV-even(j1), PV-odd(j0),PV-odd(j1) on one V set each.

Everything else follows v3: row-tiled QK pairs, ACT/DVE exp split with the
Schraudolph fp16 bit-trick, ones-column denominator inside PV, host-side
divide, early first-tile DMAs + PE pre-warm for the ramp.
"""

import os

import numpy as np

import concourse.mybir as mybir
import concourse.tile as tile
from concourse import bacc
from concourse.bass_utils import run_bass_kernel_spmd

S = 4096
H = 16
D = 64
NCORES = 8
HPC = H // NCORES  # heads per core
NKT = S // 128  # 32 k-tiles per head
NPAIR = NKT // 2  # 16 k-tile pairs (even/odd row-tiled together)
NQC = S // 512  # 8 q chunks per head
NCH = 4  # kts/qts load chunks (4 pairs / 1024 q-cols each)
SCALE = 1.0 / np.sqrt(D)

# Schraudolph fp16 exp on DVE: i16 = floor(A*s + B); bitcast to fp16.
DELTA = 0.05
A_CONST = float(1024.0 * np.log2(np.e) * SCALE)
B_CONST = float(15360.0 - 1024.0 * DELTA)

F32 = mybir.dt.float32
F16 = mybir.dt.float16
I16 = mybir.dt.int16

# slots: 2 per super-group (j = chunk parity). exp engine alternates along p
# within each chunk: 'A' = ACT exact exp, 'D' = DVE schraudolph.
def _eng(p, j):
    return "A" if (p + j) % 2 == 0 else "D"


def _phase_a(nc, sb, q, k, v, h):
    # ---- Phase A: pure-DMA loads; host ships fp16 in final layouts ----
    qts = [sb.tile([128, 1024], F16, tag=f"qt{b}", name=f"qt{b}") for b in range(NCH)]
    kts = [sb.tile([128, 512], F16, tag=f"kt{b}", name=f"kt{b}") for b in range(NCH)]
    vstage = sb.tile([128, NKT, 128], F16, tag="vstage")

    def load_v_quarter(qt):
        t0, t1 = qt * (NKT // 4), (qt + 1) * (NKT // 4)
        nc.sync.dma_start(
            vstage[:, t0:t1, :],
            v.ap()[h].rearrange("p (n c) -> p n c", c=128)[:, t0:t1],
        )

    if h == 0:
        nc.sync.dma_start(kts[0][:, 0:128], k.ap()[h, :, 0:128])
        nc.sync.dma_start(qts[0][:, 0:512], q.ap()[h, :, 0:512])
        nc.sync.dma_start(qts[0][:, 512:1024], q.ap()[h, :, 512:1024])
        nc.sync.dma_start(kts[0][:, 128:512], k.ap()[h, :, 128:512])
    else:
        nc.sync.dma_start(kts[0][:], k.ap()[h, :, 0:512])
        nc.sync.dma_start(qts[0][:], q.ap()[h, :, 0:1024])
    load_v_quarter(0)
    load_v_quarter(1)
    nc.sync.dma_start(kts[1][:], k.ap()[h, :, 512:1024])
    nc.sync.dma_start(kts[2][:], k.ap()[h, :, 1024:1536])
    nc.sync.dma_start(kts[3][:], k.ap()[h, :, 1536:2048])
    load_v_quarter(2)
    load_v_quarter(3)
    nc.sync.dma_start(qts[1][:], q.ap()[h, :, 1024:2048])
    nc.sync.dma_start(qts[2][:], q.ap()[h, :, 2048:3072])
    nc.sync.dma_start(qts[3][:], q.ap()[h, :, 3072:4096])
    return qts, kts, vstage


def _phase_b(nc, pools, tiles, o):
    sb, epool, spsum, opsum = pools

    qk_n = 256 if os.environ.get("QK_TIMING_MUTANT") else 512

    def qk_pair(h, qc, p):
        qts, kts, _ = tiles[h]
        off = (qc % 2) * 512
        b, j = p // 4, p % 4
        sp = spsum.tile([128, 1024], F32, tag="sp")
        nc.tensor.matmul(
            sp[:, 0:qk_n],
            kts[b][0:64, j * 128 : (j + 1) * 128],
            qts[qc // 2][0:64, off : off + qk_n],
            tile_position=(0, 0),
        )
        nc.tensor.matmul(
            sp[:, 512 : 512 + qk_n],
            kts[b][64:128, j * 128 : (j + 1) * 128],
            qts[qc // 2][64:128, off : off + qk_n],
            tile_position=(64, 0),
        )
        return sp

    def exp_pair(p, j, sp):
        et = epool.tile([128, 1024], F16, tag="et")
        if _eng(p, j) == "A":
            nc.scalar.activation(
                et[:], sp[:], mybir.ActivationFunctionType.Exp, scale=SCALE
            )
        else:
            nc.vector.tensor_scalar(
                et[:].bitcast(I16),
                sp[:],
                A_CONST,
                B_CONST,
                op0=mybir.AluOpType.mult,
                op1=mybir.AluOpType.add,
            )
        return et

    def pv_super(h, p, et0, et1, acc0, acc1):
        # Both chunks' PVs share each V' weight set: even(j0), even(j1),
        # odd(j0), odd(j1) — one weight-set transition per side instead of
        # one per matmul pair.
        vstage = tiles[h][2]
        for side in range(2):
            t = 2 * p + side
            for j, (et, acc) in enumerate(((et0, acc0), (et1, acc1))):
                nc.tensor.matmul(
                    acc[:],
                    vstage[:, t, :],
                    et[:, side * 512 : (side + 1) * 512],
                    start=(t == 0),
                    stop=(t == NKT - 1),
                )

    def epilogue(h, acc, qs):
        fin = sb.tile([D + 1, 512], F16, tag="fin")
        nc.scalar.copy(fin[:], acc[0 : D + 1, :])
        nc.sync.dma_start(o.ap()[h, :, qs : qs + 512], fin[:])

    # Super-slot pipeline: ss = (h, cc, p); slots 2ss+j are (chunk 2cc+j).
    NSS = HPC * (NQC // 2) * NPAIR  # 128

    def hcp(ss):
        return ss // ((NQC // 2) * NPAIR), (ss // NPAIR) % (NQC // 2), ss % NPAIR

    def qk_super(ss):
        h, cc, p = hcp(ss)
        return [qk_pair(h, 2 * cc + j, p) for j in (0, 1)]

    def exp_slot(s, sp):
        p, j = (s // 2) % NPAIR, s % 2
        return exp_pair(p, j, sp)

    sps = list(qk_super(0))
    ets = [exp_slot(0, sps[0]), exp_slot(1, sps[1])]
    sps += qk_super(1)
    accs = {}
    for ss in range(NSS):
        h, cc, p = hcp(ss)
        if p == 0:
            for j in (0, 1):
                qc = 2 * cc + j
                accs[qc] = opsum.tile([128, 512], F32, tag="acc", name=f"a{h}_{qc}")
        pv_super(h, p, ets[2 * ss], ets[2 * ss + 1], accs[2 * cc], accs[2 * cc + 1])
        if p == NPAIR - 1:
            for j in (0, 1):
                qc = 2 * cc + j
                epilogue(h, accs.pop(qc), qc * 512)
        # exps and the next QK trail the PVs in issue order so that
        # chunk-boundary epilogue copies are not queued behind them.
        if 2 * ss + 2 < 2 * NSS:
            ets.append(exp_slot(2 * ss + 2, sps[2 * ss + 2]))
        if 2 * ss + 3 < 2 * NSS:
            ets.append(exp_slot(2 * ss + 3, sps[2 * ss + 3]))
        if ss + 2 < NSS:
            sps += qk_super(ss + 2)


def _build():
    nc = bacc.Bacc(trn_type="TRN2", debug=False, num_devices=NCORES)
    q = nc.dram_tensor("q", [HPC, 128, S], F16, kind="ExternalInput")
    k = nc.dram_tensor("k", [HPC, 128, S // 2], F16, kind="ExternalInput")
    v = nc.dram_tensor("v", [HPC, 128, NKT * 128], F16, kind="ExternalInput")
    o = nc.dram_tensor("o", [HPC, D + 1, S], F16, kind="ExternalOutput")

    with tile.TileContext(nc) as tc:
        with (
            tc.tile_pool(name="const", bufs=1) as cpool,
            tc.tile_pool(name="sb", bufs=2) as sb,
            tc.tile_pool(name="epool", bufs=5) as epool,
            tc.tile_pool(name="spsum", bufs=3, space="PSUM") as spsum,
            tc.tile_pool(name="opsum", bufs=2, space="PSUM") as opsum,
        ):
            # Dummy exp pulls the ACT table-load DMA ahead of the input DMAs.
            warm = cpool.tile([128, 1], F32, tag="warm")
            nc.gpsimd.memset(warm[:], 0.0)
            nc.scalar.activation(warm[:], warm[:], mybir.ActivationFunctionType.Exp)
            # PE pre-warm (HAM clock gate) during the DMA ramp.
            wq = cpool.tile([64, 128], F16, tag="wq")
            wx = cpool.tile([64, 512], F16, tag="wx")
            nc.gpsimd.memset(wq[:], 0.0)
            nc.gpsimd.memset(wx[:], 0.0)
            warmps = spsum.tile([128, 1024], F32, tag="sp", name="warmps")
            for wi in range(12):
                nc.tensor.matmul(
                    warmps[:, (wi % 2) * 512 : (wi % 2) * 512 + 512],
                    wq[:],
                    wx[:],
                    start=True,
                    stop=True,
                )
            pools = (sb, epool, spsum, opsum)
            tiles = [_phase_a(nc, sb, q, k, v, h) for h in range(HPC)]
            _phase_b(nc, pools, tiles, o)

    nc.compile()
    return nc


_NC_CACHE = None


def _prep_inputs(query, key, value, c):
    sl = slice(c * HPC, (c + 1) * HPC)
    f16 = np.float16
    qh = query[:, sl, :].transpose(1, 2, 0).astype(f16)
    kh = key[:, sl, :].transpose(1, 2, 0).astype(f16)
    q_dup = np.concatenate([qh, qh], axis=1)
    kt = kh.reshape(HPC, D, NKT, 128)
    k_pair = np.concatenate([kt[:, :, 0::2, :], kt[:, :, 1::2, :]], axis=1).reshape(
        HPC, 128, S // 2
    )
    vh = value[:, sl, :].transpose(1, 0, 2).astype(f16)  # [HPC, S, D]
    vp = np.zeros((HPC, NKT, 128, 128), dtype=f16)
    vp[:, :, :, 0:D] = vh.reshape(HPC, NKT, 128, D)
    vp[:, :, :, D] = f16(1.0)
    v_pack = vp.transpose(0, 2, 1, 3).reshape(HPC, 128, NKT * 128)
    return {
        "q": np.ascontiguousarray(q_dup),
        "k": np.ascontiguousarray(k_pair),
        "v": np.ascontiguousarray(v_pack),
    }


def kernel(query, key, value):
    global _NC_CACHE
    if _NC_CACHE is None:
        _NC_CACHE = _build()
    nc = _NC_CACHE

    query = np.asarray(query)
    key = np.asarray(key)
    value = np.asarray(value)
    in_maps = [_prep_inputs(query, key, value, c) for c in range(NCORES)]

    res = run_bass_kernel_spmd(nc, in_maps, core_ids=list(range(NCORES)))
    outs = []
    for c in range(NCORES):
        oc = res.results[c]["o"].astype(np.float32)  # [HPC, D+1, S]
        num = oc[:, 0:D, :]
        den = oc[:, D : D + 1, :]
        outs.append((num / den).transpose(2, 0, 1))  # [S, HPC, D]
    return np.concatenate(outs, axis=1)
